# revision 1
# baseline (speedup 1.0000x reference)
"""ChebNet (K=3, 3 layers) GNN on 8 Trainium2 NeuronCores.

Strategy: node-shard across 8 cores. Per spmm: PE-transpose + dinv-scale the
shard's features to node-major bf16, ONE AllGather into a full [50000,128]
bf16 HBM table, dma_gather edge messages (edges pre-sorted by dst block on
host, self-loops excluded), scatter via one-hot PE matmuls accumulating
agg^T in PSUM, then a DVE combine that folds the self-loop diagonal in
algebraically: spmm(f) = ((A-gather-sum) + f*dinv) * dinv with degrees that
include self-loops. Dense ChebConv matmuls run feature-major on PE. All
graph preprocessing (self-loops, degrees, edge bucketing/padding, int16
index tables) happens on host inside kernel() as part of sharding.
"""
import numpy as np
import ml_dtypes
from contextlib import ExitStack

N_NODES = 50000
DIM = 128
N_LAYERS = 3
P = 8
BLK = 128
# Layer 1's input is rank-2 (w @ lin_in_w + 1 @ lin_in_b), so its two spmms
# reduce to four scalar segment-sums folded into host graph preprocessing;
# the device then starts from emb1 = relu([w,1,Aw,A1,AAw,AA1] @ C + b0).
HOST_L1 = True

_CFG_FULL = dict(N=50000, SHARD=6250)


def _preprocess(src, dst, cfg):
    N, SHARD = cfg["N"], cfg["SHARD"]
    HALF = (N + 1) // 2
    NBLK = (SHARD + BLK - 1) // BLK
    src_f = np.asarray(src, np.int64)
    dst_f = np.asarray(dst, np.int64)
    # degrees include the self-loop the reference adds
    deg = (np.bincount(dst_f, minlength=N) + 1).astype(np.float32)
    dinv = np.clip(deg, 1.0, None) ** -0.5

    owner = dst_f // SHARD
    per_core = []
    counts = np.zeros((P, NBLK, 2), np.int64)
    for c in range(P):
        m = owner == c
        s = src_f[m]
        dloc = dst_f[m] - c * SHARD
        blk = dloc // BLK
        rel = dloc % BLK
        half = (s >= HALF).astype(np.int64)
        tab = s - half * HALF
        order = np.lexsort((tab, half, blk))
        blk, rel, half, tab = blk[order], rel[order], half[order], tab[order]
        cnt = np.bincount(blk * 2 + half, minlength=NBLK * 2).reshape(NBLK, 2)
        counts[c] = cnt
        per_core.append((rel, tab, cnt))

    Tbh = np.maximum(1, -(-counts.max(axis=0) // BLK))  # [NBLK,2] tiles
    T_A, T_B = Tbh[:, 0].copy(), Tbh[:, 1].copy()
    startA = np.concatenate([[0], np.cumsum(T_A)]).astype(np.int64)
    startB = np.concatenate([[0], np.cumsum(T_B)]).astype(np.int64)
    T_totA, T_totB = int(startA[-1]), int(startB[-1])
    T_tot = T_totA + T_totB

    gidx_list, drel_list = [], []
    for c in range(P):
        rel, tab, cnt = per_core[c]
        seg = np.concatenate([[0], np.cumsum(cnt.reshape(-1))])
        idx_stream = np.zeros(T_tot * BLK, np.int64)
        rel_stream = np.full(T_tot * BLK, 255, np.int64)
        for b in range(NBLK):
            for h in (0, 1):
                n = int(cnt[b, h])
                e0 = int(seg[b * 2 + h])
                t0 = int(startA[b]) if h == 0 else T_totA + int(startB[b])
                idx_stream[t0 * BLK:t0 * BLK + n] = tab[e0:e0 + n]
                rel_stream[t0 * BLK:t0 * BLK + n] = rel[e0:e0 + n]
        pos = np.arange(T_tot * BLK)
        gw = np.zeros((128, T_tot * 8), np.int16)
        for g in range(8):
            gw[16 * g + pos % 16, pos // 16] = idx_stream
        relw = np.ascontiguousarray(
            rel_stream.reshape(T_tot, BLK).T).astype(np.float32)
        gidx_list.append(gw)
        drel_list.append(relw)

    meta = dict(N=N, SHARD=SHARD, HALF=HALF, NBLK=NBLK,
                LASTW=SHARD - (NBLK - 1) * BLK,
                T_A=T_A.tolist(), T_B=T_B.tolist(),
                startA=startA.tolist(), startB=startB.tolist(),
                T_totA=T_totA, T_totB=T_totB, T_tot=T_tot)
    return meta, dinv, gidx_list, drel_list


def _build_program(meta):
    import concourse.tile as tile
    from concourse import bacc, mybir
    f32, bf16, i16 = mybir.dt.float32, mybir.dt.bfloat16, mybir.dt.int16
    Alu, Act = mybir.AluOpType, mybir.ActivationFunctionType

    N, SHARD, HALF = meta["N"], meta["SHARD"], meta["HALF"]
    NBLK, LASTW = meta["NBLK"], meta["LASTW"]
    T_A, T_B = meta["T_A"], meta["T_B"]
    startA, startB = meta["startA"], meta["startB"]
    T_totA, T_tot = meta["T_totA"], meta["T_tot"]
    GROUPS = [(q * 4, min(4, NBLK - q * 4)) for q in range((NBLK + 3) // 4)]
    MAX_CT = 6

    def mk_calls(starts):
        calls = []
        tile2call = {}
        t_end = starts[-1]
        t = 0
        while t < t_end:
            nt = min(MAX_CT, t_end - t)
            for tt in range(t, t + nt):
                tile2call[tt] = (len(calls), tt - t)
            calls.append((t, nt))
            t += nt
        return calls, tile2call

    callsA, t2cA = mk_calls(startA)
    callsB, t2cB = mk_calls(startB)

    nc = bacc.Bacc("TRN2", target_bir_lowering=False, debug=False,
                   enable_asserts=True, num_devices=P,
                   dynamic_dma_scratch_size=24576)

    def inp(name, shape, dt):
        return nc.dram_tensor(name, shape, dt, kind="ExternalInput")

    fsh_d = inp("fsh", [8, SHARD], f32)
    cmat_d = inp("cmat", [8, 128], f32)
    dbc_d = inp("dbc", [128, SHARD], f32)
    dnode_d = inp("dnode", [128, NBLK], f32)
    gidx_d = inp("gidx", [128, T_tot * 8], i16)
    drel_d = inp("drel", [128, T_tot], f32)
    iota_d = inp("iota", [128, 128], bf16)
    ident_d = inp("ident", [128, 128], f32)
    wall_d = inp("wall", [128, N_LAYERS * 3 * 128], bf16)
    ball_d = inp("ball", [128, N_LAYERS], f32)
    predw_d = inp("predw", [128, 1], f32)
    predb_d = inp("predb", [128, 1], f32)
    out_d = nc.dram_tensor("out", [128, NBLK], f32, kind="ExternalOutput")

    ag_in = nc.dram_tensor("ag_in", [SHARD, 128], bf16)
    tab = nc.dram_tensor("tab", [N, 128], bf16, addr_space="Shared")

    with tile.TileContext(nc) as tc, ExitStack() as ctx:
        const = ctx.enter_context(tc.tile_pool(name="const", bufs=1))
        mpool = ctx.enter_context(tc.tile_pool(name="mpool", bufs=4))
        spool = ctx.enter_context(tc.tile_pool(name="spool", bufs=4))
        stagp = ctx.enter_context(tc.tile_pool(name="stagp", bufs=3))
        wpool = ctx.enter_context(tc.tile_pool(name="wpool", bufs=2))
        ps512 = ctx.enter_context(tc.tile_pool(name="ps512", bufs=2, space="PSUM"))
        psT = ctx.enter_context(tc.tile_pool(name="psT", bufs=3, space="PSUM"))

        def ld(name, dram, shape, dt):
            t = const.tile(shape, dt, tag=name)
            nc.sync.dma_start(t[:], dram.ap()[:, :])
            return t

        dbc = ld("dbc", dbc_d, [128, SHARD], f32)
        dnode = ld("dnode", dnode_d, [128, NBLK], f32)
        gidx = ld("gidx", gidx_d, [128, T_tot * 8], i16)
        drel = ld("drel", drel_d, [128, T_tot], f32)
        iota = ld("iota", iota_d, [128, 128], bf16)
        ident = ld("ident", ident_d, [128, 128], f32)
        wall = ld("wall", wall_d, [128, N_LAYERS * 3 * 128], bf16)
        ball = ld("ball", ball_d, [128, N_LAYERS], f32)
        fsh = ld("fsh", fsh_d, [8, SHARD], f32)
        cmat = ld("cmat", cmat_d, [8, 128], f32)
        predw = ld("predw", predw_d, [128, 1], f32)
        predb = ld("predb", predb_d, [128, 1], f32)

        X0f = const.tile([128, SHARD], f32, tag="X0f")
        Y1f = const.tile([128, SHARD], f32, tag="Y1f")
        AGG = const.tile([128, SHARD], f32, tag="AGG")
        X0h = const.tile([128, SHARD], bf16, tag="X0h")
        Y1h = const.tile([128, SHARD], bf16, tag="Y1h")
        X2h = const.tile([128, SHARD], bf16, tag="X2h")
        logs = const.tile([128, NBLK], f32, tag="logs")
        nc.vector.memset(logs[:], 0.0)

        regs = {}

        def nreg(v):
            if v not in regs:
                regs[v] = nc.gpsimd.to_reg(v)
            return regs[v]

        def blk_w(b):
            return BLK if b < NBLK - 1 else LASTW

        def emit_spmm(srcf, second):
            # send: PE transpose + per-node dinv scale into a per-group
            # staging tile, one batched DMA per 4-block group, one AllGather
            from concourse.ap import AP as _AP
            for q, (b0, nb) in enumerate(GROUPS):
                gst = stagp.tile([128, 512], bf16, tag="gst")
                for bi, b in enumerate(range(b0, b0 + nb)):
                    w = blk_w(b)
                    pt = psT.tile([128, 128], f32, tag="pt")
                    nc.tensor.transpose(pt[:w, :],
                                        srcf[:, b * BLK:b * BLK + w], ident[:])
                    nc.vector.tensor_scalar(gst[:w, bi * 128:(bi + 1) * 128],
                                            pt[:w, :], dnode[:w, b:b + 1],
                                            None, Alu.mult)
                nfull = nb if b0 + nb < NBLK else nb - 1
                if nfull:
                    src = gst[:, 0:nfull * 128]
                    src_ap = _AP(src.tensor, src.offset,
                                 [list(src.ap[0]), [128, nfull], [1, 128]])
                    dap = ag_in.ap()
                    dst_ap = _AP(dap.tensor, dap.offset + b0 * 16384,
                                 [[128, 128], [16384, nfull], [1, 128]])
                    nc.sync.dma_start(dst_ap, src_ap)
                if b0 + nb == NBLK:
                    b = NBLK - 1
                    nc.sync.dma_start(
                        ag_in[b * BLK:b * BLK + LASTW, :],
                        gst[:LASTW, (nb - 1) * 128:nb * 128])
            nc.gpsimd.collective_compute(
                "AllGather", Alu.bypass, replica_groups=[list(range(P))],
                ins=[ag_in.ap().opt()], outs=[tab.ap().opt()])

            # scatter: per 4-block group, both halves into one PSUM bank
            tabA, tabB = tab.ap()[0:HALF, :], tab.ap()[HALF:N, :]
            nextc = [0, 0]
            Mcall = [{}, {}]
            for q, (b0, nb) in enumerate(GROUPS):
                for h in (0, 1):
                    starts = startA if h == 0 else startB
                    calls = callsA if h == 0 else callsB
                    tbase = 0 if h == 0 else T_totA
                    tile_end = starts[b0 + nb]
                    while nextc[h] < len(calls) and \
                            calls[nextc[h]][0] < tile_end:
                        o, nt = calls[nextc[h]]
                        M = mpool.tile([128, MAX_CT, 128], bf16, tag=f"M{h}")
                        nc.gpsimd.dma_gather(
                            out_ap=M[:, :nt, :],
                            in_ap=tabA if h == 0 else tabB,
                            idxs_ap=gidx[:, 8 * (tbase + o):8 * (tbase + o + nt)],
                            num_idxs=nt * 128, num_idxs_reg=nreg(nt * 128),
                            elem_size=128)
                        Mcall[h][nextc[h]] = M
                        nextc[h] += 1
                wq = min(SHARD - b0 * BLK, nb * BLK)
                ps = ps512.tile([128, 512], f32, tag="ps")
                for bi, b in enumerate(range(b0, b0 + nb)):
                    lane = bi * 128
                    ntile = T_A[b] + T_B[b]
                    ti = 0
                    for h in (0, 1):
                        Tarr = T_A if h == 0 else T_B
                        starts = startA if h == 0 else startB
                        t2c = t2cA if h == 0 else t2cB
                        tbase = 0 if h == 0 else T_totA
                        for t in range(Tarr[b]):
                            Tg = tbase + starts[b] + t
                            cid, loc = t2c[starts[b] + t]
                            M = Mcall[h][cid]
                            S = spool.tile([128, 128], bf16, tag="S")
                            nc.vector.tensor_scalar(S[:], iota[:],
                                                    drel[:, Tg:Tg + 1], None,
                                                    Alu.is_equal)
                            nc.tensor.matmul(ps[:, lane:lane + 128],
                                             M[:, loc, :], S[:],
                                             start=(ti == 0),
                                             stop=(ti == ntile - 1))
                            ti += 1
                rng = slice(b0 * BLK, b0 * BLK + wq)
                # diag fold: spmm = (gather_sum + f*dinv) * dinv
                dg = wpool.tile([128, 512], f32, tag="dg")
                nc.vector.tensor_mul(dg[:, :wq], srcf[:, rng], dbc[:, rng])
                nc.vector.tensor_add(AGG[:, rng], ps[:, :wq], dg[:, :wq])
                if not second:
                    nc.vector.tensor_mul(Y1f[:, rng], AGG[:, rng],
                                         dbc[:, rng])
                    nc.vector.tensor_copy(Y1h[:, rng], Y1f[:, rng])
                else:
                    nc.vector.tensor_mul(AGG[:, rng], AGG[:, rng],
                                         dbc[:, rng])
                    nc.vector.scalar_tensor_tensor(
                        X2h[:, rng], AGG[:, rng], 2.0, X0f[:, rng],
                        Alu.mult, Alu.subtract)

        def emit_l1():
            # emb1 = relu(F @ C + b0): F = [w,1,Aw,A1,AAw,AA1] per node
            for q, (b0, nb) in enumerate(GROUPS):
                wq = min(SHARD - b0 * BLK, nb * BLK)
                rng = slice(b0 * BLK, b0 * BLK + wq)
                ps = ps512.tile([128, 512], f32, tag="ps")
                nc.tensor.matmul(ps[:, :wq], cmat[0:6, :], fsh[0:6, rng],
                                 start=True, stop=True)
                nc.scalar.activation(X0f[:, rng], ps[:, :wq], Act.Relu,
                                     bias=ball[:, 0:1])
                nc.vector.tensor_copy(X0h[:, rng], X0f[:, rng])

        def emit_dense(layer):
            terms = [X0h, Y1h, X2h]
            for q, (b0, nb) in enumerate(GROUPS):
                wq = min(SHARD - b0 * BLK, nb * BLK)
                rng = slice(b0 * BLK, b0 * BLK + wq)
                ps = ps512.tile([128, 512], f32, tag="ps")
                for k in range(3):
                    c0 = (3 * layer + k) * 128
                    nc.tensor.matmul(ps[:, :wq], wall[:, c0:c0 + 128],
                                     terms[k][:, rng], start=(k == 0),
                                     stop=(k == 2))
                nc.scalar.activation(X0f[:, rng], ps[:, :wq], Act.Relu,
                                     bias=ball[:, layer:layer + 1])
                nc.vector.tensor_copy(X0h[:, rng], X0f[:, rng])

        def emit_pred():
            for b in range(NBLK):
                w = blk_w(b)
                ps = psT.tile([128, 128], f32, tag="pp")
                nc.tensor.matmul(ps[:w, 0:1], X0f[:, b * BLK:b * BLK + w],
                                 predw[:], start=True, stop=True)
                nc.scalar.activation(logs[:w, b:b + 1], ps[:w, 0:1],
                                     Act.Identity, bias=predb[:w, 0:1])
            nc.sync.dma_start(out_d.ap()[:, :], logs[:])

        emit_l1()
        for layer in range(1, N_LAYERS):
            emit_spmm(X0f, second=False)
            emit_spmm(Y1f, second=True)
            emit_dense(layer)
        emit_pred()

    nc.compile()
    return nc


def _host_fields(inputs, dinv, n):
    """[w, 1, Aw, A1, AAw, AA1] per node plus the collapsed layer-1 input
    matrix C[6,128]: relu(F @ C + b0) == ChebConv_0(w @ lin_w + lin_b)."""
    w = np.asarray(inputs["weights"], np.float64)
    src = np.asarray(inputs["src"])
    dst = np.asarray(inputs["dst"])
    dv = dinv.astype(np.float64)

    def spmm(v):
        h = v * dv
        agg = np.bincount(dst, weights=h[src], minlength=n) + h
        return agg * dv

    one = np.ones(n, np.float64)
    a1, b1 = spmm(w), spmm(one)
    a2, b2 = spmm(a1), spmm(b1)
    F = np.stack([w, one, a1, b1, a2, b2,
                  np.zeros(n), np.zeros(n)]).astype(np.float32)  # [8, N]

    lw = np.asarray(inputs["lin_in_w"], np.float64).reshape(1, 128)
    lb = np.asarray(inputs["lin_in_b"], np.float64).reshape(1, 128)
    W = np.asarray(inputs["cheb_ws"], np.float64)[0]
    W0, W1, W2 = W[0:128], W[128:256], W[256:384]
    C = np.concatenate([
        lw @ (W0 - W2), lb @ (W0 - W2),
        -lw @ W1, -lb @ W1,
        2.0 * (lw @ W2), 2.0 * (lb @ W2),
        np.zeros((2, 128)),
    ]).astype(np.float32)  # [8, 128]
    return F, C


def _in_maps(inputs, meta, dinv, gidx_list, drel_list):
    N, SHARD = meta["N"], meta["SHARD"]
    NBLK = meta["NBLK"]

    weights = np.asarray(inputs["weights"], np.float32)
    lin_in_w = np.asarray(inputs["lin_in_w"], np.float32)
    lin_in_b = np.asarray(inputs["lin_in_b"], np.float32)
    cheb_ws = np.asarray(inputs["cheb_ws"], np.float32)
    cheb_bs = np.asarray(inputs["cheb_bs"], np.float32)
    pred_w = np.asarray(inputs["pred_w"], np.float32)
    pred_b = np.asarray(inputs["pred_b"], np.float32)
    if HOST_L1:
        F, C = _host_fields(inputs, dinv, N)

    iota = np.broadcast_to(np.arange(128, dtype=np.float32), (128, 128))
    iota = np.ascontiguousarray(iota).astype(ml_dtypes.bfloat16)
    ident = np.eye(128, dtype=np.float32)
    wall = np.zeros((128, N_LAYERS * 3 * 128), np.float32)
    for l in range(N_LAYERS):
        for k in range(3):
            w = cheb_ws[l][k * 128:(k + 1) * 128, :]
            wall[:, (3 * l + k) * 128:(3 * l + k + 1) * 128] = \
                -w if k == 1 else w
    wall = wall.astype(ml_dtypes.bfloat16)
    ball = np.ascontiguousarray(cheb_bs.T).astype(np.float32)
    shared = dict(
        iota=iota, ident=ident, wall=wall, ball=ball,
        predw=pred_w.reshape(128, 1).astype(np.float32),
        predb=np.full((128, 1), float(pred_b[0]), np.float32),
    )
    if HOST_L1:
        shared["cmat"] = C
    else:
        shared["linw"] = np.ascontiguousarray(lin_in_w.reshape(1, 128).T)
        shared["linb"] = lin_in_b.reshape(128, 1).astype(np.float32)
    in_maps = []
    for c in range(P):
        dv = dinv[c * SHARD:(c + 1) * SHARD]
        dn = np.ones(NBLK * BLK, np.float32)
        dn[:SHARD] = dv
        dn = np.ascontiguousarray(dn.reshape(NBLK, BLK).T)
        m = dict(shared)
        if HOST_L1:
            m["fsh"] = np.ascontiguousarray(
                F[:, c * SHARD:(c + 1) * SHARD])
        else:
            wsh = weights[c * SHARD:(c + 1) * SHARD]
            m["wbc"] = np.ascontiguousarray(
                np.broadcast_to(wsh, (128, SHARD))).astype(np.float32)
        m["dbc"] = np.ascontiguousarray(
            np.broadcast_to(dv, (128, SHARD))).astype(np.float32)
        m["dnode"] = dn
        m["gidx"] = gidx_list[c]
        m["drel"] = drel_list[c]
        in_maps.append(m)
    return in_maps


def _run(inputs, cfg, trace=False, time_runs=0):
    import time
    from concourse.bass_utils import run_bass_kernel_spmd
    SHARD = cfg["SHARD"]

    src = np.asarray(inputs["src"])
    dst = np.asarray(inputs["dst"])
    meta, dinv, gidx_list, drel_list = _preprocess(src, dst, cfg)
    nc = _build_program(meta)
    in_maps = _in_maps(inputs, meta, dinv, gidx_list, drel_list)

    res = run_bass_kernel_spmd(nc, in_maps, core_ids=list(range(P)),
                               trace=trace)
    extra = {"run_walls": []}
    for _ in range(time_runs):
        t0 = time.time()
        run_bass_kernel_spmd(nc, in_maps, core_ids=list(range(P)),
                             trace=False)
        extra["run_walls"].append(time.time() - t0)
    parts = []
    for c in range(P):
        o = res.results[c]["out"]  # [128, NBLK]
        parts.append(np.ascontiguousarray(o.T).reshape(-1)[:SHARD])
    logits = np.concatenate(parts).astype(np.float32)[:, None]
    return logits, res, extra


def kernel(**inputs):
    logits, _, _ = _run(inputs, _CFG_FULL, trace=False)
    return logits



# revision 4
# speedup vs baseline: 1.5921x; 1.5921x over previous
"""ChebNet (K=3, 3 layers) GNN on 8 Trainium2 NeuronCores.

Strategy: node-shard across 8 cores. Per spmm: PE-transpose + dinv-scale the
shard's features to node-major bf16, ONE AllGather into a full [50000,128]
bf16 HBM table, dma_gather edge messages (edges pre-sorted by dst block on
host, self-loops excluded), scatter via one-hot PE matmuls accumulating
agg^T in PSUM, then a DVE combine that folds the self-loop diagonal in
algebraically: spmm(f) = ((A-gather-sum) + f*dinv) * dinv with degrees that
include self-loops. Dense ChebConv matmuls run feature-major on PE. All
graph preprocessing (self-loops, degrees, edge bucketing/padding, int16
index tables) happens on host inside kernel() as part of sharding.

Host->device traffic is minimized (the axon tunnel is ~20MB/s): the dinv
broadcast matrices are derived on device from one [1,SHARD] vector via
stride-0 / strided DMAs, gather indices ship in 16-partition form and are
replicated on device, rel-slot ids ship as uint8, iota/identity are
generated with gpsimd.iota, and the dense weights ship as per-core 1/8
slices that an AllGather reassembles.
"""
import numpy as np
import ml_dtypes
from contextlib import ExitStack

N_NODES = 50000
DIM = 128
N_LAYERS = 3
P = 8
BLK = 128
# Layer 1's input is rank-2 (w @ lin_in_w + 1 @ lin_in_b), so its two spmms
# reduce to four scalar segment-sums folded into host graph preprocessing;
# the device then starts from emb1 = relu([w,1,Aw,A1,AAw,AA1] @ C + b0).
HOST_L1 = True

_CFG_FULL = dict(N=50000, SHARD=6250)


def _preprocess(src, dst, cfg):
    N, SHARD = cfg["N"], cfg["SHARD"]
    HALF = (N + 1) // 2
    NBLK = (SHARD + BLK - 1) // BLK
    src_f = np.asarray(src, np.int64)
    dst_f = np.asarray(dst, np.int64)
    # degrees include the self-loop the reference adds
    deg = (np.bincount(dst_f, minlength=N) + 1).astype(np.float32)
    dinv = np.clip(deg, 1.0, None) ** -0.5

    owner = dst_f // SHARD
    per_core = []
    counts = np.zeros((P, NBLK, 2), np.int64)
    for c in range(P):
        m = owner == c
        s = src_f[m]
        dloc = dst_f[m] - c * SHARD
        blk = dloc // BLK
        rel = dloc % BLK
        half = (s >= HALF).astype(np.int64)
        tab = s - half * HALF
        order = np.lexsort((tab, half, blk))
        blk, rel, half, tab = blk[order], rel[order], half[order], tab[order]
        cnt = np.bincount(blk * 2 + half, minlength=NBLK * 2).reshape(NBLK, 2)
        counts[c] = cnt
        per_core.append((rel, tab, cnt))

    Tbh = np.maximum(1, -(-counts.max(axis=0) // BLK))  # [NBLK,2] tiles
    T_A, T_B = Tbh[:, 0].copy(), Tbh[:, 1].copy()
    startA = np.concatenate([[0], np.cumsum(T_A)]).astype(np.int64)
    startB = np.concatenate([[0], np.cumsum(T_B)]).astype(np.int64)
    T_totA, T_totB = int(startA[-1]), int(startB[-1])
    T_tot = T_totA + T_totB

    gidx_list, drel_list = [], []
    for c in range(P):
        rel, tab, cnt = per_core[c]
        seg = np.concatenate([[0], np.cumsum(cnt.reshape(-1))])
        idx_stream = np.zeros(T_tot * BLK, np.int64)
        rel_stream = np.full(T_tot * BLK, 255, np.int64)
        for b in range(NBLK):
            for h in (0, 1):
                n = int(cnt[b, h])
                e0 = int(seg[b * 2 + h])
                t0 = int(startA[b]) if h == 0 else T_totA + int(startB[b])
                idx_stream[t0 * BLK:t0 * BLK + n] = tab[e0:e0 + n]
                rel_stream[t0 * BLK:t0 * BLK + n] = rel[e0:e0 + n]
        pos = np.arange(T_tot * BLK)
        gw16 = np.zeros((16, T_tot * 8), np.int16)
        gw16[pos % 16, pos // 16] = idx_stream
        rel8 = np.ascontiguousarray(
            rel_stream.reshape(T_tot, BLK).T).astype(np.uint8)
        gidx_list.append(gw16)
        drel_list.append(rel8)

    meta = dict(N=N, SHARD=SHARD, HALF=HALF, NBLK=NBLK,
                LASTW=SHARD - (NBLK - 1) * BLK,
                T_A=T_A.tolist(), T_B=T_B.tolist(),
                startA=startA.tolist(), startB=startB.tolist(),
                T_totA=T_totA, T_totB=T_totB, T_tot=T_tot)
    return meta, dinv, gidx_list, drel_list


def _build_program(meta):
    import concourse.tile as tile
    from concourse import bacc, mybir
    from concourse.ap import AP as _AP
    f32, bf16, i16 = mybir.dt.float32, mybir.dt.bfloat16, mybir.dt.int16
    u8 = mybir.dt.uint8
    Alu, Act = mybir.AluOpType, mybir.ActivationFunctionType

    N, SHARD, HALF = meta["N"], meta["SHARD"], meta["HALF"]
    NBLK, LASTW = meta["NBLK"], meta["LASTW"]
    T_A, T_B = meta["T_A"], meta["T_B"]
    startA, startB = meta["startA"], meta["startB"]
    T_totA, T_tot = meta["T_totA"], meta["T_tot"]
    GROUPS = [(q * 4, min(4, NBLK - q * 4)) for q in range((NBLK + 3) // 4)]
    MAX_CT = 6

    def mk_calls(starts):
        calls = []
        tile2call = {}
        t_end = starts[-1]
        t = 0
        while t < t_end:
            nt = min(MAX_CT, t_end - t)
            for tt in range(t, t + nt):
                tile2call[tt] = (len(calls), tt - t)
            calls.append((t, nt))
            t += nt
        return calls, tile2call

    callsA, t2cA = mk_calls(startA)
    callsB, t2cB = mk_calls(startB)

    nc = bacc.Bacc("TRN2", target_bir_lowering=False, debug=False,
                   enable_asserts=True, num_devices=P,
                   dynamic_dma_scratch_size=24576)

    def inp(name, shape, dt):
        return nc.dram_tensor(name, shape, dt, kind="ExternalInput")

    fsh_d = inp("fsh", [6, SHARD], f32)
    cmat_d = inp("cmat", [6, 128], f32)
    dinvr_d = inp("dinvr", [1, NBLK * BLK], f32)
    gidx16_d = inp("gidx16", [16, T_tot * 8], i16)
    drel8_d = inp("drel8", [128, T_tot], u8)
    wall16_d = inp("wall16", [16, N_LAYERS * 3 * 128], bf16)
    ball_d = inp("ball", [128, N_LAYERS], f32)
    predw_d = inp("predw", [128, 1], f32)
    predb_d = inp("predb", [128, 1], f32)
    out_d = nc.dram_tensor("out", [128, NBLK], f32, kind="ExternalOutput")

    ag_in = nc.dram_tensor("ag_in", [SHARD, 128], bf16)
    tab = nc.dram_tensor("tab", [N, 128], bf16, addr_space="Shared")
    wall_s = nc.dram_tensor("wall_s", [16, N_LAYERS * 3 * 128], bf16)
    wall_g = nc.dram_tensor("wall_g", [128, N_LAYERS * 3 * 128], bf16,
                            addr_space="Shared")

    with tile.TileContext(nc) as tc, ExitStack() as ctx:
        const = ctx.enter_context(tc.tile_pool(name="const", bufs=1))
        mpool = ctx.enter_context(tc.tile_pool(name="mpool", bufs=4))
        spool = ctx.enter_context(tc.tile_pool(name="spool", bufs=4))
        stagp = ctx.enter_context(tc.tile_pool(name="stagp", bufs=3))
        wpool = ctx.enter_context(tc.tile_pool(name="wpool", bufs=2))
        ps512 = ctx.enter_context(tc.tile_pool(name="ps512", bufs=2, space="PSUM"))
        psT = ctx.enter_context(tc.tile_pool(name="psT", bufs=3, space="PSUM"))

        def ld(name, dram, shape, dt, engine=None):
            t = const.tile(shape, dt, tag=name)
            (engine or nc.sync).dma_start(t[:], dram.ap()[:, :])
            return t

        # dinv broadcast [128, SHARD]: stride-0 partition-broadcast DMA
        dbc = const.tile([128, SHARD], f32, tag="dbc")
        dsrc = dinvr_d.ap()
        nc.sync.dma_start(dbc[:], _AP(dsrc.tensor, dsrc.offset,
                                      [[0, 128], [1, SHARD]]))
        # dinv node-major [128, NBLK]: strided DMA (elem (r,b) <- dinvr[b*128+r])
        dnode = const.tile([128, NBLK], f32, tag="dnode")
        nc.sync.dma_start(dnode[:], _AP(dsrc.tensor, dsrc.offset,
                                        [[1, 128], [128, NBLK]]))
        # gather indices: replicate [16, T*8] across the 8 gpsimd cores
        gidx = const.tile([128, T_tot * 8], i16, tag="gidx")
        for g in range(8):
            nc.sync.dma_start(gidx[16 * g:16 * (g + 1), :],
                              gidx16_d.ap()[:, :])
        # rel-slot ids: uint8 -> f32 on device
        drel8 = ld("drel8", drel8_d, [128, T_tot], u8)
        drel = const.tile([128, T_tot], f32, tag="drel")
        nc.vector.tensor_copy(drel[:], drel8[:])
        # iota (bf16 row 0..127) and identity (f32) generated on device
        coli = const.tile([128, 128], i16, tag="coli")
        nc.gpsimd.iota(coli[:], pattern=[[1, 128]], base=0,
                       channel_multiplier=0)
        pidx = const.tile([128, 1], i16, tag="pidx")
        nc.gpsimd.iota(pidx[:], pattern=[[0, 1]], base=0,
                       channel_multiplier=1)
        iota = const.tile([128, 128], bf16, tag="iota")
        nc.vector.tensor_copy(iota[:], coli[:])
        colf = const.tile([128, 128], f32, tag="colf")
        nc.vector.tensor_copy(colf[:], coli[:])
        pidxf = const.tile([128, 1], f32, tag="pidxf")
        nc.vector.tensor_copy(pidxf[:], pidx[:])
        ident = const.tile([128, 128], f32, tag="ident")
        nc.vector.tensor_scalar(ident[:], colf[:], pidxf[:, 0:1], None,
                                Alu.is_equal)
        # dense weights: each core ships rows [16c:16c+16]; AllGather
        # (collectives cannot read IO tensors -> stage through wall_s)
        nc.sync.dma_start(wall_s.ap()[:, :], wall16_d.ap()[:, :])
        nc.gpsimd.collective_compute(
            "AllGather", Alu.bypass, replica_groups=[list(range(P))],
            ins=[wall_s.ap().opt()], outs=[wall_g.ap().opt()])
        wall = const.tile([128, N_LAYERS * 3 * 128], bf16, tag="wall")
        nc.gpsimd.dma_start(wall[:], wall_g.ap()[:, :])

        ball = ld("ball", ball_d, [128, N_LAYERS], f32)
        fsh = ld("fsh", fsh_d, [6, SHARD], f32)
        cmat = ld("cmat", cmat_d, [6, 128], f32)
        predw = ld("predw", predw_d, [128, 1], f32)
        predb = ld("predb", predb_d, [128, 1], f32)

        X0f = const.tile([128, SHARD], f32, tag="X0f")
        Y1f = const.tile([128, SHARD], f32, tag="Y1f")
        AGG = const.tile([128, SHARD], f32, tag="AGG")
        X0h = const.tile([128, SHARD], bf16, tag="X0h")
        Y1h = const.tile([128, SHARD], bf16, tag="Y1h")
        X2h = const.tile([128, SHARD], bf16, tag="X2h")
        logs = const.tile([128, NBLK], f32, tag="logs")
        nc.vector.memset(logs[:], 0.0)

        regs = {}

        def nreg(v):
            if v not in regs:
                regs[v] = nc.gpsimd.to_reg(v)
            return regs[v]

        def blk_w(b):
            return BLK if b < NBLK - 1 else LASTW

        def emit_spmm(srcf, second):
            # send: PE transpose + per-node dinv scale into a per-group
            # staging tile, one batched DMA per 4-block group, one AllGather
            for q, (b0, nb) in enumerate(GROUPS):
                gst = stagp.tile([128, 512], bf16, tag="gst")
                for bi, b in enumerate(range(b0, b0 + nb)):
                    w = blk_w(b)
                    pt = psT.tile([128, 128], f32, tag="pt")
                    nc.tensor.transpose(pt[:w, :],
                                        srcf[:, b * BLK:b * BLK + w], ident[:])
                    nc.vector.tensor_scalar(gst[:w, bi * 128:(bi + 1) * 128],
                                            pt[:w, :], dnode[:w, b:b + 1],
                                            None, Alu.mult)
                nfull = nb if b0 + nb < NBLK else nb - 1
                if nfull:
                    src = gst[:, 0:nfull * 128]
                    src_ap = _AP(src.tensor, src.offset,
                                 [list(src.ap[0]), [128, nfull], [1, 128]])
                    dap = ag_in.ap()
                    dst_ap = _AP(dap.tensor, dap.offset + b0 * 16384,
                                 [[128, 128], [16384, nfull], [1, 128]])
                    nc.sync.dma_start(dst_ap, src_ap)
                if b0 + nb == NBLK:
                    b = NBLK - 1
                    nc.sync.dma_start(
                        ag_in[b * BLK:b * BLK + LASTW, :],
                        gst[:LASTW, (nb - 1) * 128:nb * 128])
            nc.gpsimd.collective_compute(
                "AllGather", Alu.bypass, replica_groups=[list(range(P))],
                ins=[ag_in.ap().opt()], outs=[tab.ap().opt()])

            # scatter: per 4-block group, both halves into one PSUM bank
            tabA, tabB = tab.ap()[0:HALF, :], tab.ap()[HALF:N, :]
            nextc = [0, 0]
            Mcall = [{}, {}]
            for q, (b0, nb) in enumerate(GROUPS):
                for h in (0, 1):
                    starts = startA if h == 0 else startB
                    calls = callsA if h == 0 else callsB
                    tbase = 0 if h == 0 else T_totA
                    tile_end = starts[b0 + nb]
                    while nextc[h] < len(calls) and \
                            calls[nextc[h]][0] < tile_end:
                        o, nt = calls[nextc[h]]
                        M = mpool.tile([128, MAX_CT, 128], bf16, tag=f"M{h}")
                        nc.gpsimd.dma_gather(
                            out_ap=M[:, :nt, :],
                            in_ap=tabA if h == 0 else tabB,
                            idxs_ap=gidx[:, 8 * (tbase + o):8 * (tbase + o + nt)],
                            num_idxs=nt * 128, num_idxs_reg=nreg(nt * 128),
                            elem_size=128)
                        Mcall[h][nextc[h]] = M
                        nextc[h] += 1
                wq = min(SHARD - b0 * BLK, nb * BLK)
                ps = ps512.tile([128, 512], f32, tag="ps")
                for bi, b in enumerate(range(b0, b0 + nb)):
                    lane = bi * 128
                    ntile = T_A[b] + T_B[b]
                    ti = 0
                    for h in (0, 1):
                        Tarr = T_A if h == 0 else T_B
                        starts = startA if h == 0 else startB
                        t2c = t2cA if h == 0 else t2cB
                        tbase = 0 if h == 0 else T_totA
                        for t in range(Tarr[b]):
                            Tg = tbase + starts[b] + t
                            cid, loc = t2c[starts[b] + t]
                            M = Mcall[h][cid]
                            S = spool.tile([128, 128], bf16, tag="S")
                            nc.vector.tensor_scalar(S[:], iota[:],
                                                    drel[:, Tg:Tg + 1], None,
                                                    Alu.is_equal)
                            nc.tensor.matmul(ps[:, lane:lane + 128],
                                             M[:, loc, :], S[:],
                                             start=(ti == 0),
                                             stop=(ti == ntile - 1))
                            ti += 1
                rng = slice(b0 * BLK, b0 * BLK + wq)
                # diag fold: spmm = (gather_sum + f*dinv) * dinv
                dg = wpool.tile([128, 512], f32, tag="dg")
                nc.vector.tensor_mul(dg[:, :wq], srcf[:, rng], dbc[:, rng])
                nc.vector.tensor_add(AGG[:, rng], ps[:, :wq], dg[:, :wq])
                if not second:
                    nc.vector.tensor_mul(Y1f[:, rng], AGG[:, rng],
                                         dbc[:, rng])
                    nc.vector.tensor_copy(Y1h[:, rng], Y1f[:, rng])
                else:
                    nc.vector.tensor_mul(AGG[:, rng], AGG[:, rng],
                                         dbc[:, rng])
                    nc.vector.scalar_tensor_tensor(
                        X2h[:, rng], AGG[:, rng], 2.0, X0f[:, rng],
                        Alu.mult, Alu.subtract)

        def emit_l1():
            # emb1 = relu(F @ C + b0): F = [w,1,Aw,A1,AAw,AA1] per node
            for q, (b0, nb) in enumerate(GROUPS):
                wq = min(SHARD - b0 * BLK, nb * BLK)
                rng = slice(b0 * BLK, b0 * BLK + wq)
                ps = ps512.tile([128, 512], f32, tag="ps")
                nc.tensor.matmul(ps[:, :wq], cmat[:], fsh[:, rng],
                                 start=True, stop=True)
                nc.scalar.activation(X0f[:, rng], ps[:, :wq], Act.Relu,
                                     bias=ball[:, 0:1])
                nc.vector.tensor_copy(X0h[:, rng], X0f[:, rng])

        def emit_dense(layer):
            terms = [X0h, Y1h, X2h]
            for q, (b0, nb) in enumerate(GROUPS):
                wq = min(SHARD - b0 * BLK, nb * BLK)
                rng = slice(b0 * BLK, b0 * BLK + wq)
                ps = ps512.tile([128, 512], f32, tag="ps")
                for k in range(3):
                    c0 = (3 * layer + k) * 128
                    nc.tensor.matmul(ps[:, :wq], wall[:, c0:c0 + 128],
                                     terms[k][:, rng], start=(k == 0),
                                     stop=(k == 2))
                nc.scalar.activation(X0f[:, rng], ps[:, :wq], Act.Relu,
                                     bias=ball[:, layer:layer + 1])
                nc.vector.tensor_copy(X0h[:, rng], X0f[:, rng])

        def emit_pred():
            for b in range(NBLK):
                w = blk_w(b)
                ps = psT.tile([128, 128], f32, tag="pp")
                nc.tensor.matmul(ps[:w, 0:1], X0f[:, b * BLK:b * BLK + w],
                                 predw[:], start=True, stop=True)
                nc.scalar.activation(logs[:w, b:b + 1], ps[:w, 0:1],
                                     Act.Identity, bias=predb[:w, 0:1])
            nc.sync.dma_start(out_d.ap()[:, :], logs[:])

        emit_l1()
        for layer in range(1, N_LAYERS):
            emit_spmm(X0f, second=False)
            emit_spmm(Y1f, second=True)
            emit_dense(layer)
        emit_pred()

    nc.compile()
    return nc


def _host_fields(inputs, dinv, n):
    """[w, 1, Aw, A1, AAw, AA1] per node plus the collapsed layer-1 input
    matrix C[6,128]: relu(F @ C + b0) == ChebConv_0(w @ lin_w + lin_b)."""
    w = np.asarray(inputs["weights"], np.float64)
    src = np.asarray(inputs["src"])
    dst = np.asarray(inputs["dst"])
    dv = dinv.astype(np.float64)

    def spmm(v):
        h = v * dv
        agg = np.bincount(dst, weights=h[src], minlength=n) + h
        return agg * dv

    one = np.ones(n, np.float64)
    a1, b1 = spmm(w), spmm(one)
    a2, b2 = spmm(a1), spmm(b1)
    F = np.stack([w, one, a1, b1, a2, b2]).astype(np.float32)  # [6, N]

    lw = np.asarray(inputs["lin_in_w"], np.float64).reshape(1, 128)
    lb = np.asarray(inputs["lin_in_b"], np.float64).reshape(1, 128)
    W = np.asarray(inputs["cheb_ws"], np.float64)[0]
    W0, W1, W2 = W[0:128], W[128:256], W[256:384]
    C = np.concatenate([
        lw @ (W0 - W2), lb @ (W0 - W2),
        -lw @ W1, -lb @ W1,
        2.0 * (lw @ W2), 2.0 * (lb @ W2),
    ]).astype(np.float32)  # [6, 128]
    return F, C


def _in_maps(inputs, meta, dinv, gidx_list, drel_list):
    N, SHARD = meta["N"], meta["SHARD"]
    NBLK = meta["NBLK"]

    cheb_ws = np.asarray(inputs["cheb_ws"], np.float32)
    cheb_bs = np.asarray(inputs["cheb_bs"], np.float32)
    pred_w = np.asarray(inputs["pred_w"], np.float32)
    pred_b = np.asarray(inputs["pred_b"], np.float32)
    F, C = _host_fields(inputs, dinv, N)

    wall = np.zeros((128, N_LAYERS * 3 * 128), np.float32)
    for l in range(N_LAYERS):
        for k in range(3):
            w = cheb_ws[l][k * 128:(k + 1) * 128, :]
            wall[:, (3 * l + k) * 128:(3 * l + k + 1) * 128] = \
                -w if k == 1 else w
    wall = wall.astype(ml_dtypes.bfloat16)
    ball = np.ascontiguousarray(cheb_bs.T).astype(np.float32)
    shared = dict(
        ball=ball, cmat=C,
        predw=pred_w.reshape(128, 1).astype(np.float32),
        predb=np.full((128, 1), float(pred_b[0]), np.float32),
    )
    in_maps = []
    for c in range(P):
        dv = dinv[c * SHARD:(c + 1) * SHARD]
        dn = np.ones(NBLK * BLK, np.float32)
        dn[:SHARD] = dv
        m = dict(shared)
        m["fsh"] = np.ascontiguousarray(F[:, c * SHARD:(c + 1) * SHARD])
        m["dinvr"] = dn.reshape(1, -1)
        m["gidx16"] = gidx_list[c]
        m["drel8"] = drel_list[c]
        m["wall16"] = np.ascontiguousarray(wall[16 * c:16 * (c + 1), :])
        in_maps.append(m)
    return in_maps


def _run(inputs, cfg, trace=False, time_runs=0):
    import time
    from concourse.bass_utils import run_bass_kernel_spmd
    SHARD = cfg["SHARD"]

    src = np.asarray(inputs["src"])
    dst = np.asarray(inputs["dst"])
    meta, dinv, gidx_list, drel_list = _preprocess(src, dst, cfg)
    nc = _build_program(meta)
    in_maps = _in_maps(inputs, meta, dinv, gidx_list, drel_list)

    res = run_bass_kernel_spmd(nc, in_maps, core_ids=list(range(P)),
                               trace=trace)
    extra = {"run_walls": []}
    for _ in range(time_runs):
        t0 = time.time()
        run_bass_kernel_spmd(nc, in_maps, core_ids=list(range(P)),
                             trace=False)
        extra["run_walls"].append(time.time() - t0)
    parts = []
    for c in range(P):
        o = res.results[c]["out"]  # [128, NBLK]
        parts.append(np.ascontiguousarray(o.T).reshape(-1)[:SHARD])
    logits = np.concatenate(parts).astype(np.float32)[:, None]
    return logits, res, extra


def kernel(**inputs):
    logits, _, _ = _run(inputs, _CFG_FULL, trace=False)
    return logits


# revision 27
# speedup vs baseline: 4.6895x; 2.9454x over previous
"""ChebNet (K=3, 3 layers) GNN on 8 Trainium2 NeuronCores.

Strategy: node-shard across 8 cores. Per spmm: PE-transpose + dinv-scale the
shard's features to node-major bf16, ONE AllGather into a full [50000,128]
bf16 HBM table, dma_gather edge messages (edges pre-sorted by dst block on
host, self-loops excluded), scatter via one-hot PE matmuls accumulating
agg^T in PSUM, then a DVE combine that folds the self-loop diagonal in
algebraically: spmm(f) = ((A-gather-sum) + f*dinv) * dinv with degrees that
include self-loops. Dense ChebConv matmuls run feature-major on PE. All
graph preprocessing (self-loops, degrees, edge bucketing/padding, int16
index tables) happens on host inside kernel() as part of sharding.

The repeated-run wall cost is dominated by per-call framework work
(axon input upload at ~20MB/s and a walrus recompile of the BIR on every
fresh jit), so the kernel is built to minimize both:
 - host->device bytes: dinv broadcast matrices are derived on device from
   one [1,SHARD] vector via stride-0 / strided DMAs, gather indices ship in
   16-partition form and are replicated on device, rel-slot ids ship as
   uint8, iota/identity are generated with gpsimd.iota, and the dense
   weights ship as per-core 1/8 slices that an AllGather reassembles.
 - BIR size: every per-block stage (send transpose/scale, gather, one-hot
   scatter matmuls, combine, dense, pred) runs in tc.For_i hardware loops
   over uniform 4-block groups with ds() dynamic slices; blocks are padded
   to a uniform tile count per half so the loop body is static.
"""
import numpy as np
import ml_dtypes
from contextlib import ExitStack

N_NODES = 50000
DIM = 128
N_LAYERS = 3
P = 8
BLK = 128
# Layer 1's input is rank-2 (w @ lin_in_w + 1 @ lin_in_b), so its two spmms
# reduce to four scalar segment-sums folded into host graph preprocessing;
# the device then starts from emb1 = relu([w,1,Aw,A1,AAw,AA1] @ C + b0).
HOST_L1 = True

_CFG_FULL = dict(N=50000, SHARD=6250)


def _preprocess(src, dst, cfg):
    N, SHARD = cfg["N"], cfg["SHARD"]
    HALF = (N + 1) // 2
    NBLK = (SHARD + BLK - 1) // BLK
    src_f = np.asarray(src, np.int64)
    dst_f = np.asarray(dst, np.int64)
    # degrees include the self-loop the reference adds
    deg = (np.bincount(dst_f, minlength=N) + 1).astype(np.float32)
    dinv = np.clip(deg, 1.0, None) ** -0.5

    owner = dst_f // SHARD
    per_core = []
    counts = np.zeros((P, NBLK, 2), np.int64)
    for c in range(P):
        m = owner == c
        s = src_f[m]
        dloc = dst_f[m] - c * SHARD
        blk = dloc // BLK
        rel = dloc % BLK
        half = (s >= HALF).astype(np.int64)
        tab = s - half * HALF
        order = np.lexsort((tab, half, blk))
        blk, rel, half, tab = blk[order], rel[order], half[order], tab[order]
        cnt = np.bincount(blk * 2 + half, minlength=NBLK * 2).reshape(NBLK, 2)
        counts[c] = cnt
        per_core.append((rel, tab, cnt))

    # uniform tiles per (block, half): pad every block-half to the max so
    # the scatter runs in a hardware loop with a static body
    Tbh = np.maximum(1, -(-counts.max(axis=0) // BLK))  # [NBLK,2]
    T_mA, T_mB = int(Tbh[:, 0].max()), int(Tbh[:, 1].max())
    T_totA = NBLK * T_mA
    T_tot = NBLK * (T_mA + T_mB)

    gidx_list, drel_list = [], []
    for c in range(P):
        rel, tab, cnt = per_core[c]
        seg = np.concatenate([[0], np.cumsum(cnt.reshape(-1))])
        idx_stream = np.zeros(T_tot * BLK, np.int64)
        rel_stream = np.full(T_tot * BLK, 255, np.int64)
        for b in range(NBLK):
            for h in (0, 1):
                n = int(cnt[b, h])
                e0 = int(seg[b * 2 + h])
                t0 = b * T_mA if h == 0 else T_totA + b * T_mB
                idx_stream[t0 * BLK:t0 * BLK + n] = tab[e0:e0 + n]
                rel_stream[t0 * BLK:t0 * BLK + n] = rel[e0:e0 + n]
        pos = np.arange(T_tot * BLK)
        gw16 = np.zeros((16, T_tot * 8), np.int16)
        gw16[pos % 16, pos // 16] = idx_stream
        rel8 = np.ascontiguousarray(
            rel_stream.reshape(T_tot, BLK).T).astype(np.uint8)
        gidx_list.append(gw16)
        drel_list.append(rel8)

    meta = dict(N=N, SHARD=SHARD, HALF=HALF, NBLK=NBLK,
                LASTW=SHARD - (NBLK - 1) * BLK,
                T_mA=T_mA, T_mB=T_mB, T_totA=T_totA, T_tot=T_tot)
    return meta, dinv, gidx_list, drel_list


def _build_program(meta):
    import concourse.tile as tile
    from concourse import bacc, mybir
    from concourse.ap import AP as _AP
    from concourse.bass import ds
    f32, bf16, i16 = mybir.dt.float32, mybir.dt.bfloat16, mybir.dt.int16
    u8 = mybir.dt.uint8
    Alu, Act = mybir.AluOpType, mybir.ActivationFunctionType

    N, SHARD, HALF = meta["N"], meta["SHARD"], meta["HALF"]
    NBLK, LASTW = meta["NBLK"], meta["LASTW"]
    T_mA, T_mB = meta["T_mA"], meta["T_mB"]
    T_totA, T_tot = meta["T_totA"], meta["T_tot"]
    NG = NBLK // 4          # full 4-block groups in hardware loops
    assert NBLK == NG * 4 + 1  # last block peeled (width LASTW)
    TPC = 6                 # tiles per dma_gather call (hard HW limit ~8)

    nc = bacc.Bacc("TRN2", target_bir_lowering=False, debug=False,
                   enable_asserts=True, num_devices=P,
                   dynamic_dma_scratch_size=24576)

    def inp(name, shape, dt):
        return nc.dram_tensor(name, shape, dt, kind="ExternalInput")

    fsh_d = inp("fsh", [6, SHARD], f32)
    cmat_d = inp("cmat", [6, 128], f32)
    dinvr_d = inp("dinvr", [1, NBLK * BLK], f32)
    gidx16_d = inp("gidx16", [16, T_tot * 8], i16)
    drel8_d = inp("drel8", [128, T_tot], u8)
    wall16_d = inp("wall16", [16, N_LAYERS * 3 * 128], bf16)
    ball_d = inp("ball", [128, N_LAYERS], f32)
    predw_d = inp("predw", [128, 1], f32)
    predb_d = inp("predb", [128, 1], f32)
    out_d = nc.dram_tensor("out", [128, NBLK], f32, kind="ExternalOutput")

    import os as _os0
    dum_d = (inp("dum", [N, 128], bf16)
             if _os0.environ.get("KDUM") == "1" else None)
    ag_in = nc.dram_tensor("ag_in", [SHARD, 128], bf16)
    tab = nc.dram_tensor("tab", [N, 128], bf16, addr_space="Shared")
    wall_s = nc.dram_tensor("wall_s", [16, N_LAYERS * 3 * 128], bf16)
    wall_g = nc.dram_tensor("wall_g", [128, N_LAYERS * 3 * 128], bf16,
                            addr_space="Shared")

    with tile.TileContext(nc) as tc, ExitStack() as ctx:
        const = ctx.enter_context(tc.tile_pool(name="const", bufs=1))
        mpool = ctx.enter_context(tc.tile_pool(name="mpool", bufs=4))
        spool = ctx.enter_context(tc.tile_pool(name="spool", bufs=4))
        stg32 = ctx.enter_context(tc.tile_pool(name="stg32", bufs=2))
        stagp = ctx.enter_context(tc.tile_pool(name="stagp", bufs=2))
        wpool = ctx.enter_context(tc.tile_pool(name="wpool", bufs=2))
        ps512 = ctx.enter_context(tc.tile_pool(name="ps512", bufs=2, space="PSUM"))
        psT = ctx.enter_context(tc.tile_pool(name="psT", bufs=3, space="PSUM"))

        def ld(name, dram, shape, dt):
            t = const.tile(shape, dt, tag=name)
            nc.sync.dma_start(t[:], dram.ap()[:, :])
            return t

        # dinv broadcast [128, SHARD]: stride-0 partition-broadcast DMA
        dbc = const.tile([128, SHARD], f32, tag="dbc")
        dsrc = dinvr_d.ap()
        nc.sync.dma_start(dbc[:], _AP(dsrc.tensor, dsrc.offset,
                                      [[0, 128], [1, SHARD]]))
        # dinv node-major [128, NBLK]: strided DMA (elem (r,b) <- dinvr[b*128+r])
        dnode = const.tile([128, NBLK], f32, tag="dnode")
        nc.sync.dma_start(dnode[:], _AP(dsrc.tensor, dsrc.offset,
                                        [[1, 128], [128, NBLK]]))
        # gather indices: replicate [16, T*8] across the 8 gpsimd cores
        gidx = const.tile([128, T_tot * 8], i16, tag="gidx")
        for g in range(8):
            nc.sync.dma_start(gidx[16 * g:16 * (g + 1), :],
                              gidx16_d.ap()[:, :])
        # rel-slot ids: uint8 -> f32 on device
        drel8 = ld("drel8", drel8_d, [128, T_tot], u8)
        drel = const.tile([128, T_tot], f32, tag="drel")
        nc.vector.tensor_copy(drel[:], drel8[:])
        # iota (bf16 row 0..127) and identity (f32) generated on device
        coli = const.tile([128, 128], i16, tag="coli")
        nc.gpsimd.iota(coli[:], pattern=[[1, 128]], base=0,
                       channel_multiplier=0)
        pidx = const.tile([128, 1], i16, tag="pidx")
        nc.gpsimd.iota(pidx[:], pattern=[[0, 1]], base=0,
                       channel_multiplier=1)
        iota = const.tile([128, 128], bf16, tag="iota")
        nc.vector.tensor_copy(iota[:], coli[:])
        colf = const.tile([128, 128], f32, tag="colf")
        nc.vector.tensor_copy(colf[:], coli[:])
        pidxf = const.tile([128, 1], f32, tag="pidxf")
        nc.vector.tensor_copy(pidxf[:], pidx[:])
        ident = const.tile([128, 128], f32, tag="ident")
        nc.vector.tensor_scalar(ident[:], colf[:], pidxf[:, 0:1], None,
                                Alu.is_equal)
        # dense weights: each core ships rows [16c:16c+16]; AllGather
        # (collectives cannot read IO tensors -> stage through wall_s)
        nc.sync.dma_start(wall_s.ap()[:, :], wall16_d.ap()[:, :])
        nc.gpsimd.collective_compute(
            "AllGather", Alu.bypass, replica_groups=[list(range(P))],
            ins=[wall_s.ap().opt()], outs=[wall_g.ap().opt()])
        wall = const.tile([128, N_LAYERS * 3 * 128], bf16, tag="wall")
        nc.gpsimd.dma_start(wall[:], wall_g.ap()[:, :])

        ball = ld("ball", ball_d, [128, N_LAYERS], f32)
        fsh = ld("fsh", fsh_d, [6, SHARD], f32)
        cmat = ld("cmat", cmat_d, [6, 128], f32)
        predw = ld("predw", predw_d, [128, 1], f32)
        predb = ld("predb", predb_d, [128, 1], f32)

        X0f = const.tile([128, SHARD], f32, tag="X0f")
        Y1f = const.tile([128, SHARD], f32, tag="Y1f")
        X0h = const.tile([128, SHARD], bf16, tag="X0h")
        Y1h = const.tile([128, SHARD], bf16, tag="Y1h")
        X2h = const.tile([128, SHARD], bf16, tag="X2h")
        logs = const.tile([128, NBLK], f32, tag="logs")
        nc.vector.memset(logs[:], 0.0)

        regs = {}

        def nreg(v):
            if v not in regs:
                regs[v] = nc.gpsimd.to_reg(v)
            return regs[v]

        # pre-build gather-count registers OUTSIDE the hardware loops
        for n in (4 * T_mA, 4 * T_mB, T_mA, T_mB):
            for o in range(0, n, TPC):
                nreg(min(TPC, n - o) * 128)

        if dum_d is not None:
            tabA, tabB = dum_d.ap()[0:HALF, :], dum_d.ap()[HALF:N, :]
        else:
            tabA, tabB = tab.ap()[0:HALF, :], tab.ap()[HALF:N, :]

        def scatter_tiles(ps, lane, mA, mB, colA, colB):
            """one block: T_mA+T_mB one-hot matmuls accumulating in ps."""
            ntile = T_mA + T_mB
            ti = 0
            for T_m, mf, col in ((T_mA, mA, colA), (T_mB, mB, colB)):
                for t in range(T_m):
                    S = spool.tile([128, 128], bf16, tag="S")
                    nc.vector.tensor_scalar(S[:], iota[:], col(t), None,
                                            Alu.is_equal)
                    nc.tensor.matmul(ps[:, lane:lane + 128], mf(t), S[:],
                                     start=(ti == 0), stop=(ti == ntile - 1))
                    ti += 1

        def combine(srcf, second, rng, wq, ps):
            # diag fold: spmm = (gather_sum + f*dinv) * dinv
            dg = wpool.tile([128, 512], f32, tag="dg")
            ag = wpool.tile([128, 512], f32, tag="ag")
            nc.vector.tensor_mul(dg[:, :wq], srcf[:, rng], dbc[:, rng])
            nc.vector.tensor_add(ag[:, :wq], ps[:, :wq], dg[:, :wq])
            if not second:
                nc.vector.tensor_mul(Y1f[:, rng], ag[:, :wq], dbc[:, rng])
                nc.vector.tensor_copy(Y1h[:, rng], Y1f[:, rng])
            else:
                nc.vector.tensor_mul(ag[:, :wq], ag[:, :wq], dbc[:, rng])
                nc.vector.scalar_tensor_tensor(
                    X2h[:, rng], ag[:, :wq], 2.0, X0f[:, rng],
                    Alu.mult, Alu.subtract)

        def emit_spmm(srcf, second):
            # send: stage 4 blocks, PE transpose + per-node dinv scale,
            # one DMA per group into ag_in, then one AllGather
            with tc.For_i(0, NG) as q:
                stage = stg32.tile([128, 512], f32, tag="stage")
                nc.vector.tensor_copy(stage[:], srcf[:, ds(q * 512, 512)])
                gst = stagp.tile([128, 512], bf16, tag="gst")
                for bi in range(4):
                    pt = psT.tile([128, 128], f32, tag="pt")
                    nc.tensor.transpose(pt[:], stage[:, bi * 128:(bi + 1) * 128],
                                        ident[:])
                    nc.vector.tensor_scalar(gst[:, bi * 128:(bi + 1) * 128],
                                            pt[:], dnode[:, ds(q * 4 + bi, 1)],
                                            None, Alu.mult)
                # gst[p, bi*128+f] holds node (q*512 + bi*128 + p), feature f:
                # 3D transpose-aware DMA (p, bi, f) -> ag_in row q*512+bi*128+p
                src = gst[:, 0:512]
                src_ap = _AP(src.tensor, src.offset,
                             [list(src.ap[0]), [128, 4], [1, 128]])
                dbase = ag_in[ds(q * 512, 512), :]
                dst_ap = _AP(dbase.tensor, dbase.offset,
                             [[128, 128], [16384, 4], [1, 128]])
                nc.sync.dma_start(dst_ap, src_ap)
            b = NBLK - 1
            pt = psT.tile([128, 128], f32, tag="pt")
            nc.tensor.transpose(pt[:LASTW, :],
                                srcf[:, b * BLK:b * BLK + LASTW], ident[:])
            gst = stagp.tile([128, 128], bf16, tag="gst2")
            nc.vector.tensor_scalar(gst[:LASTW, :], pt[:LASTW, :],
                                    dnode[:LASTW, b:b + 1], None, Alu.mult)
            nc.sync.dma_start(ag_in[b * BLK:b * BLK + LASTW, :],
                              gst[:LASTW, :])
            nc.gpsimd.collective_compute(
                "AllGather", Alu.bypass, replica_groups=[list(range(P))],
                ins=[ag_in.ap().opt()], outs=[tab.ap().opt()])

            import os as _os
            if _os.environ.get("KSPART") == "1":   # send+AG only: fake spmm
                nc.vector.tensor_copy(Y1f[:], srcf[:])
                nc.vector.tensor_copy(Y1h[:], Y1f[:])
                if second:
                    nc.vector.tensor_copy(X2h[:], srcf[:])
                return

            # scatter: 4 blocks per iteration, both halves, one PSUM bank.
            # dma_gather calls are capped at TPC=6 tiles (768 indices) and
            # rotate a 4-buffer pool -- larger calls crash the gpsimd
            # descriptor machinery (hard limit found empirically between
            # 1024 and 1536 indices per call).
            nA, nB = 4 * T_mA, 4 * T_mB
            callsA = [(o, min(TPC, nA - o)) for o in range(0, nA, TPC)]
            callsB = [(o, min(TPC, nB - o)) for o in range(0, nB, TPC)]
            with tc.For_i(0, NG) as q:
                Ms = {}
                for ci, (o, nt) in enumerate(callsA):
                    M = mpool.tile([128, TPC, 128], bf16, tag="MA")
                    nc.gpsimd.dma_gather(
                        out_ap=M[:, :nt, :], in_ap=tabA,
                        idxs_ap=gidx[:, ds(q * (nA * 8) + o * 8, nt * 8)],
                        num_idxs=nt * 128, num_idxs_reg=nreg(nt * 128),
                        elem_size=128)
                    Ms[(0, ci)] = M
                for ci, (o, nt) in enumerate(callsB):
                    M = mpool.tile([128, TPC, 128], bf16, tag="MB")
                    nc.gpsimd.dma_gather(
                        out_ap=M[:, :nt, :], in_ap=tabB,
                        idxs_ap=gidx[:, ds(T_totA * 8 + q * (nB * 8) + o * 8,
                                           nt * 8)],
                        num_idxs=nt * 128, num_idxs_reg=nreg(nt * 128),
                        elem_size=128)
                    Ms[(1, ci)] = M
                ps = ps512.tile([128, 512], f32, tag="ps")
                for bi in range(4):
                    scatter_tiles(
                        ps, bi * 128,
                        lambda t, bi=bi: Ms[(0, (bi * T_mA + t) // TPC)]
                        [:, (bi * T_mA + t) % TPC, :],
                        lambda t, bi=bi: Ms[(1, (bi * T_mB + t) // TPC)]
                        [:, (bi * T_mB + t) % TPC, :],
                        lambda t, bi=bi: drel[:, ds(q * (4 * T_mA)
                                                    + (bi * T_mA + t), 1)],
                        lambda t, bi=bi: drel[:, ds(q * (4 * T_mB)
                                                    + (T_totA + bi * T_mB
                                                       + t), 1)])
                combine(srcf, second, ds(q * 512, 512), 512, ps)
            # peeled last block (width LASTW)
            b = NBLK - 1
            Msp = {}
            for ci, (o, nt) in enumerate(
                    [(o, min(TPC, T_mA - o)) for o in range(0, T_mA, TPC)]):
                M = mpool.tile([128, TPC, 128], bf16, tag="MA")
                nc.gpsimd.dma_gather(
                    out_ap=M[:, :nt, :], in_ap=tabA,
                    idxs_ap=gidx[:, 8 * (b * T_mA + o):
                                 8 * (b * T_mA + o + nt)],
                    num_idxs=nt * 128, num_idxs_reg=nreg(nt * 128),
                    elem_size=128)
                Msp[(0, ci)] = M
            for ci, (o, nt) in enumerate(
                    [(o, min(TPC, T_mB - o)) for o in range(0, T_mB, TPC)]):
                M = mpool.tile([128, TPC, 128], bf16, tag="MB")
                nc.gpsimd.dma_gather(
                    out_ap=M[:, :nt, :], in_ap=tabB,
                    idxs_ap=gidx[:, 8 * (T_totA + b * T_mB + o):
                                 8 * (T_totA + b * T_mB + o + nt)],
                    num_idxs=nt * 128, num_idxs_reg=nreg(nt * 128),
                    elem_size=128)
                Msp[(1, ci)] = M
            ps = ps512.tile([128, 512], f32, tag="ps")
            scatter_tiles(
                ps, 0,
                lambda t: Msp[(0, t // TPC)][:, t % TPC, :],
                lambda t: Msp[(1, t // TPC)][:, t % TPC, :],
                lambda t: drel[:, b * T_mA + t:b * T_mA + t + 1],
                lambda t: drel[:, T_totA + b * T_mB + t:
                               T_totA + b * T_mB + t + 1])
            combine(srcf, second, slice(b * BLK, b * BLK + LASTW), LASTW, ps)

        def emit_l1():
            # emb1 = relu(F @ C + b0): F = [w,1,Aw,A1,AAw,AA1] per node
            with tc.For_i(0, NG) as q:
                rng = ds(q * 512, 512)
                ps = ps512.tile([128, 512], f32, tag="ps")
                nc.tensor.matmul(ps[:], cmat[:], fsh[:, rng],
                                 start=True, stop=True)
                nc.scalar.activation(X0f[:, rng], ps[:], Act.Relu,
                                     bias=ball[:, 0:1])
                nc.vector.tensor_copy(X0h[:, rng], X0f[:, rng])
            b = NBLK - 1
            rng = slice(b * BLK, b * BLK + LASTW)
            ps = ps512.tile([128, 512], f32, tag="ps")
            nc.tensor.matmul(ps[:, :LASTW], cmat[:], fsh[:, rng],
                             start=True, stop=True)
            nc.scalar.activation(X0f[:, rng], ps[:, :LASTW], Act.Relu,
                                 bias=ball[:, 0:1])
            nc.vector.tensor_copy(X0h[:, rng], X0f[:, rng])

        def emit_dense(layer):
            terms = [X0h, Y1h, X2h]

            def body(rng, wq):
                ps = ps512.tile([128, 512], f32, tag="ps")
                for k in range(3):
                    c0 = (3 * layer + k) * 128
                    nc.tensor.matmul(ps[:, :wq], wall[:, c0:c0 + 128],
                                     terms[k][:, rng], start=(k == 0),
                                     stop=(k == 2))
                nc.scalar.activation(X0f[:, rng], ps[:, :wq], Act.Relu,
                                     bias=ball[:, layer:layer + 1])
                nc.vector.tensor_copy(X0h[:, rng], X0f[:, rng])

            with tc.For_i(0, NG) as q:
                body(ds(q * 512, 512), 512)
            b = NBLK - 1
            body(slice(b * BLK, b * BLK + LASTW), LASTW)

        def emit_pred():
            with tc.For_i(0, NG) as q:
                stage = stg32.tile([128, 512], f32, tag="pst")
                nc.vector.tensor_copy(stage[:], X0f[:, ds(q * 512, 512)])
                for bi in range(4):
                    ps = psT.tile([128, 128], f32, tag="pp")
                    nc.tensor.matmul(ps[:, 0:1],
                                     stage[:, bi * 128:(bi + 1) * 128],
                                     predw[:], start=True, stop=True)
                    nc.vector.tensor_scalar(logs[:, ds(q * 4 + bi, 1)],
                                            ps[:, 0:1], predb[:, 0:1], None,
                                            Alu.add)
            b = NBLK - 1
            ps = psT.tile([128, 128], f32, tag="pp")
            nc.tensor.matmul(ps[:LASTW, 0:1], X0f[:, b * BLK:b * BLK + LASTW],
                             predw[:], start=True, stop=True)
            nc.scalar.activation(logs[:LASTW, b:b + 1], ps[:LASTW, 0:1],
                                 Act.Identity, bias=predb[:LASTW, 0:1])
            nc.sync.dma_start(out_d.ap()[:, :], logs[:])

        import os as _os
        _stage = float(_os.environ.get("KSTAGE", "99"))
        if _stage == 0:
            nc.sync.dma_start(out_d.ap()[:, :], logs[:])
        elif _stage == 0.7:     # setup + l1 only
            emit_l1()
            nc.sync.dma_start(out_d.ap()[:, :], logs[:])
        elif _stage == 0.8:     # setup + pred only
            nc.vector.memset(X0f[:], 0.25)
            emit_pred()
        else:
            emit_l1()
            if _stage >= 2:
                for layer in range(1, N_LAYERS):
                    emit_spmm(X0f, second=False)
                    emit_spmm(Y1f, second=True)
                    emit_dense(layer)
                    if _stage < 3:
                        break
            emit_pred()

    nc.compile()
    return nc


def _host_fields(inputs, dinv, n):
    """[w, 1, Aw, A1, AAw, AA1] per node plus the collapsed layer-1 input
    matrix C[6,128]: relu(F @ C + b0) == ChebConv_0(w @ lin_w + lin_b)."""
    w = np.asarray(inputs["weights"], np.float64)
    src = np.asarray(inputs["src"])
    dst = np.asarray(inputs["dst"])
    dv = dinv.astype(np.float64)

    def spmm(v):
        h = v * dv
        agg = np.bincount(dst, weights=h[src], minlength=n) + h
        return agg * dv

    one = np.ones(n, np.float64)
    a1, b1 = spmm(w), spmm(one)
    a2, b2 = spmm(a1), spmm(b1)
    F = np.stack([w, one, a1, b1, a2, b2]).astype(np.float32)  # [6, N]

    lw = np.asarray(inputs["lin_in_w"], np.float64).reshape(1, 128)
    lb = np.asarray(inputs["lin_in_b"], np.float64).reshape(1, 128)
    W = np.asarray(inputs["cheb_ws"], np.float64)[0]
    W0, W1, W2 = W[0:128], W[128:256], W[256:384]
    C = np.concatenate([
        lw @ (W0 - W2), lb @ (W0 - W2),
        -lw @ W1, -lb @ W1,
        2.0 * (lw @ W2), 2.0 * (lb @ W2),
    ]).astype(np.float32)  # [6, 128]
    return F, C


def _in_maps(inputs, meta, dinv, gidx_list, drel_list):
    N, SHARD = meta["N"], meta["SHARD"]
    NBLK = meta["NBLK"]

    cheb_ws = np.asarray(inputs["cheb_ws"], np.float32)
    cheb_bs = np.asarray(inputs["cheb_bs"], np.float32)
    pred_w = np.asarray(inputs["pred_w"], np.float32)
    pred_b = np.asarray(inputs["pred_b"], np.float32)
    F, C = _host_fields(inputs, dinv, N)

    wall = np.zeros((128, N_LAYERS * 3 * 128), np.float32)
    for l in range(N_LAYERS):
        for k in range(3):
            w = cheb_ws[l][k * 128:(k + 1) * 128, :]
            wall[:, (3 * l + k) * 128:(3 * l + k + 1) * 128] = \
                -w if k == 1 else w
    wall = wall.astype(ml_dtypes.bfloat16)
    ball = np.ascontiguousarray(cheb_bs.T).astype(np.float32)
    shared = dict(
        ball=ball, cmat=C,
        predw=pred_w.reshape(128, 1).astype(np.float32),
        predb=np.full((128, 1), float(pred_b[0]), np.float32),
    )
    import os as _os
    if _os.environ.get("KDUM") == "1":
        shared["dum"] = np.zeros((N, 128), ml_dtypes.bfloat16)
    in_maps = []
    for c in range(P):
        dv = dinv[c * SHARD:(c + 1) * SHARD]
        dn = np.ones(NBLK * BLK, np.float32)
        dn[:SHARD] = dv
        m = dict(shared)
        m["fsh"] = np.ascontiguousarray(F[:, c * SHARD:(c + 1) * SHARD])
        m["dinvr"] = dn.reshape(1, -1)
        m["gidx16"] = gidx_list[c]
        m["drel8"] = drel_list[c]
        m["wall16"] = np.ascontiguousarray(wall[16 * c:16 * (c + 1), :])
        in_maps.append(m)
    return in_maps


def _run(inputs, cfg, trace=False, time_runs=0):
    import time
    from concourse.bass_utils import run_bass_kernel_spmd
    SHARD = cfg["SHARD"]

    src = np.asarray(inputs["src"])
    dst = np.asarray(inputs["dst"])
    meta, dinv, gidx_list, drel_list = _preprocess(src, dst, cfg)
    nc = _build_program(meta)
    in_maps = _in_maps(inputs, meta, dinv, gidx_list, drel_list)

    res = run_bass_kernel_spmd(nc, in_maps, core_ids=list(range(P)),
                               trace=trace)
    extra = {"run_walls": []}
    for _ in range(time_runs):
        t0 = time.time()
        run_bass_kernel_spmd(nc, in_maps, core_ids=list(range(P)),
                             trace=False)
        extra["run_walls"].append(time.time() - t0)
    parts = []
    for c in range(P):
        o = res.results[c]["out"]  # [128, NBLK]
        parts.append(np.ascontiguousarray(o.T).reshape(-1)[:SHARD])
    logits = np.concatenate(parts).astype(np.float32)[:, None]
    return logits, res, extra


def kernel(**inputs):
    logits, _, _ = _run(inputs, _CFG_FULL, trace=False)
    return logits
